# revision 40
# baseline (speedup 1.0000x reference)
"""Trainium2 Bass kernel for CrossMultiheadAttention.

B=4, T=S=1024, E=1024, H=16, D=64. 8 NeuronCores.

Sharding: core c handles (batch b=c//2, T-half th=c%2) -> 512 query rows.
Each core computes k/v projections for its whole batch (duplicated between
the 2 cores sharing a batch), all 16 heads of attention for its queries and
the full output projection for its rows. Output gather is a pure concat.

v2: the PE only reaches its 2.4GHz p-state when the instruction stream is
continuously busy (it idles back to 1.2GHz otherwise), so the kernel is
organized as one long gapless matmul stream:
  - Q/K/V projections run et-major over 4 concurrent PSUM targets so the
    first matmuls issue as soon as the first weight/input tiles land.
  - The second half of the K/V projections (s in [512,1024) for K, ih=1 for
    V) is emitted as filler bursts inside the attention loop, exactly
    covering the cycles where attention alone would leave the PE idle
    behind DVE/ACT.
  - Attention keeps the baseline dataflow (scoresT [s,t] with a fused
    ones-column denominator in the attn@v matmul) but both heads of a pair
    share one 2-bank PSUM tile so bias-add and exp cover 1024 columns per
    instruction; bias streams in bf16; adds are split DVE/Pool.
  - Normalization is per head pair with no DRAM roundtrip: approx
    reciprocal straight off the PSUM denominator row, broadcast via a tiny
    fp32 matmul, one multiply to produce the bf16 out-proj input.
  - Out-projection consumes unormalized... normalized oT directly after the
    last head pair; output DMA is spread across the phase.
"""
import sys

sys.path.insert(0, "/opt/trn_rl_repo")

import numpy as np
import ml_dtypes

import concourse.bass as bass
import concourse.bacc as bacc
import concourse.tile as tile
from concourse import mybir
from concourse.bass_utils import run_bass_kernel_spmd


def _pbcast(ap, nparts):
    """View `ap` (a [1, N] row) replicated across nparts partitions via a
    0-stride partition dim — DMA-source only."""
    row = ap
    return bass.AP(tensor=row.tensor, offset=row.offset,
                   ap=[[0, nparts]] + [list(d) for d in row.ap[1:]])

F32 = mybir.dt.float32
BF16 = mybir.dt.bfloat16
Act = mybir.ActivationFunctionType
Alu = mybir.AluOpType
NPBF16 = ml_dtypes.bfloat16

B, T, S, E, H, D = 4, 1024, 1024, 1024, 16, 64
HP = H // 2          # head pairs
TS = T // 2          # per-core query rows (t-shard)
ET = E // 128        # 128-row tiles of the embed dim
SCALING = D ** -0.5
MASK_NEG = -10000.0

_CACHE = {}


def build_nc():
    nc = bacc.Bacc("TRN2", target_bir_lowering=False, debug=False, num_devices=8)

    qin_d = nc.dram_tensor("qin", [E, TS], BF16, kind="ExternalInput").ap()
    kin_d = nc.dram_tensor("kin", [E, S], BF16, kind="ExternalInput").ap()
    vin_d = nc.dram_tensor("vin", [E, S], BF16, kind="ExternalInput").ap()
    bias_d = nc.dram_tensor("biasT", [H, S, TS], BF16, kind="ExternalInput").ap()
    mask_d = nc.dram_tensor("maskneg", [128, 8], F32, kind="ExternalInput").ap()
    wq_d = nc.dram_tensor("wqt", [E, E], BF16, kind="ExternalInput").ap()
    wk_d = nc.dram_tensor("wkt", [E, E], BF16, kind="ExternalInput").ap()
    wv_d = nc.dram_tensor("wvt", [E, E], BF16, kind="ExternalInput").ap()
    wo_d = nc.dram_tensor("wot", [E, E], BF16, kind="ExternalInput").ap()
    bq_d = nc.dram_tensor("bqs", [128, 8], F32, kind="ExternalInput").ap()
    bk_d = nc.dram_tensor("bks", [128, 8], F32, kind="ExternalInput").ap()
    bv_d = nc.dram_tensor("bvr", [1, E], BF16, kind="ExternalInput").ap()
    bo_d = nc.dram_tensor("bor", [1, E], BF16, kind="ExternalInput").ap()
    id_d = nc.dram_tensor("ident", [128, 128], BF16, kind="ExternalInput").ap()
    out_d = nc.dram_tensor("out", [TS, E], F32, kind="ExternalOutput").ap()

    with tile.TileContext(nc) as tc:
        with tc.tile_pool(name="consts", bufs=1) as consts, \
             tc.tile_pool(name="wpool", bufs=32) as wpool, \
             tc.tile_pool(name="kvin", bufs=24) as kvin, \
             tc.tile_pool(name="persist", bufs=1) as persist, \
             tc.tile_pool(name="btp", bufs=5) as btp, \
             tc.tile_pool(name="eip", bufs=3) as eip, \
             tc.tile_pool(name="ep", bufs=7) as ep, \
             tc.tile_pool(name="small", bufs=2) as small, \
             tc.tile_pool(name="osbp", bufs=2) as osbp, \
             tc.tile_pool(name="dramp", bufs=2, space="DRAM") as dramp, \
             tc.tile_pool(name="ps", bufs=2, space="PSUM") as psp:

            # ================= DMA preamble (need-time ordered) ===========
            wq_sb, qin_sb = [], []
            for et in range(ET):
                w = wpool.tile([128, E], BF16, tag="w", name="w")
                nc.sync.dma_start(out=w, in_=wq_d[et * 128:(et + 1) * 128, :])
                wq_sb.append(w)
                t_ = kvin.tile([128, TS], BF16, tag="qin", name="qin", bufs=8)
                nc.sync.dma_start(out=t_, in_=qin_d[et * 128:(et + 1) * 128, :])
                qin_sb.append(t_)

            mask_sb = consts.tile([128, 8], F32, tag="mask")
            bq_sb = consts.tile([128, 8], F32, tag="bq")
            bk_sb = consts.tile([128, 8], F32, tag="bk")
            nc.sync.dma_start(out=mask_sb, in_=mask_d)
            nc.sync.dma_start(out=bq_sb, in_=bq_d)
            nc.sync.dma_start(out=bk_sb, in_=bk_d)

            wk_sb, kin_sb = [], []
            for et in range(ET):
                w = wpool.tile([128, E], BF16, tag="w", name="w")
                nc.sync.dma_start(out=w, in_=wk_d[et * 128:(et + 1) * 128, :])
                wk_sb.append(w)
                t_ = kvin.tile([128, S], BF16, tag="kin", name="kin", bufs=8)
                nc.sync.dma_start(out=t_[:, 0:512],
                                  in_=kin_d[et * 128:(et + 1) * 128, 0:512])
                kin_sb.append(t_)

            wv_sb, vin_sb = [], []
            for et in range(ET):
                w = wpool.tile([128, E], BF16, tag="w", name="w")
                nc.sync.dma_start(out=w, in_=wv_d[et * 128:(et + 1) * 128, :])
                wv_sb.append(w)
                t_ = kvin.tile([128, S], BF16, tag="vin", name="vin", bufs=8)
                nc.sync.dma_start(out=t_[:, 0:512],
                                  in_=vin_d[et * 128:(et + 1) * 128, 0:512])
                vin_sb.append(t_)
            for et in range(ET):
                nc.sync.dma_start(out=vin_sb[et][:, 512:1024],
                                  in_=vin_d[et * 128:(et + 1) * 128, 512:1024])
            for et in range(ET):
                nc.sync.dma_start(out=kin_sb[et][:, 512:1024],
                                  in_=kin_d[et * 128:(et + 1) * 128,
                                            512:1024])

            bvb = consts.tile([128, E], BF16, tag="bvb")
            bob = consts.tile([128, E], BF16, tag="bob")
            nc.sync.dma_start(out=bvb, in_=_pbcast(bv_d, 128))
            nc.sync.dma_start(out=bob, in_=_pbcast(bo_d, 128))

            ident = consts.tile([128, 128], BF16, tag="ident")
            nc.sync.dma_start(out=ident, in_=id_d)

            # ================= persistent products =========================
            qT = [persist.tile([128, TS], BF16, tag=f"qt{hp}", name=f"qt{hp}")
                  for hp in range(HP)]
            kT = [[persist.tile([128, 512], BF16, tag=f"kt{hp}_{sh}",
                                name=f"kt{hp}_{sh}")
                   for sh in range(2)] for hp in range(HP)]
            v65 = [persist.tile([128, H, 65], BF16, tag=f"v65_{j}",
                                name=f"v65_{j}")
                   for j in range(8)]
            otn2 = [persist.tile([128, TS], BF16, tag=f"otn2_{et}",
                                 name=f"otn2_{et}")
                    for et in range(ET)]
            for j in range(8):
                nc.gpsimd.memset(v65[j][:, :, 64:65], 1.0)

            def pair_tile():
                return psp.tile([128, 2, 512], F32, tag="pair", name="pair",
                                bufs=2)

            def aux_tile():
                return psp.tile([128, 512], F32, tag="aux", name="aux",
                                bufs=2)

            # ============ Q projection: et-major, 2 groups of 4 hp ========
            for g in range(2):
                pa, pb = pair_tile(), pair_tile()
                halves = [pa[:, 0, :], pa[:, 1, :], pb[:, 0, :], pb[:, 1, :]]
                for et in range(ET):
                    for i4 in range(4):
                        hp = 4 * g + i4
                        nc.tensor.matmul(
                            halves[i4], wq_sb[et][:, hp * 128:(hp + 1) * 128],
                            qin_sb[et], start=(et == 0), stop=(et == ET - 1))
                for i4 in range(4):
                    hp = 4 * g + i4
                    nc.scalar.activation(qT[hp], halves[i4], Act.Identity,
                                         bias=bq_sb[:, hp:hp + 1])

            # ============ K projection sh=0: et-major =====================
            for g in range(2):
                pa, pb = pair_tile(), pair_tile()
                halves = [pa[:, 0, :], pa[:, 1, :], pb[:, 0, :], pb[:, 1, :]]
                for et in range(ET):
                    for i4 in range(4):
                        hp = 4 * g + i4
                        nc.tensor.matmul(
                            halves[i4], wk_sb[et][:, hp * 128:(hp + 1) * 128],
                            kin_sb[et][:, 0:512],
                            start=(et == 0), stop=(et == ET - 1))
                for i4 in range(4):
                    hp = 4 * g + i4
                    nc.scalar.activation(kT[hp][0], halves[i4], Act.Identity,
                                         bias=bk_sb[:, hp:hp + 1])

            # ============ V projection sh=0 (8 targets) + sh=1 ih=0 =======
            def v_group(targets):
                # targets: list of (sh, st, ih)
                tiles = [pair_tile() for _ in range((len(targets) + 1) // 2)]
                halves = []
                for i, tgt in enumerate(targets):
                    halves.append(tiles[i // 2][:, i % 2, :])
                for et in range(ET):
                    for i, (sh, st, ih) in enumerate(targets):
                        nc.tensor.matmul(
                            halves[i],
                            vin_sb[et][:, sh * 512 + st * 128:
                                       sh * 512 + (st + 1) * 128],
                            wv_sb[et][:, ih * 512:(ih + 1) * 512],
                            start=(et == 0), stop=(et == ET - 1))
                for i, (sh, st, ih) in enumerate(targets):
                    j = sh * 4 + st
                    nc.vector.tensor_tensor(
                        out=v65[j][:, ih * 8:(ih + 1) * 8, 0:64],
                        in0=halves[i].rearrange("p (h d) -> p h d", h=8),
                        in1=bvb[:, ih * 512:(ih + 1) * 512].rearrange(
                            "p (h d) -> p h d", h=8),
                        op=Alu.add)

            v_group([(0, 0, 0), (0, 0, 1), (0, 1, 0), (0, 1, 1)])
            v_group([(0, 2, 0), (0, 2, 1), (0, 3, 0), (0, 3, 1)])
            v_group([(1, 0, 0), (1, 1, 0), (1, 2, 0), (1, 3, 0)])

            # ============ K projection sh=1 hp=0 (pre-attention) ==========
            def k_sh1_burst(hp):
                ops = []
                t_ = aux_tile()
                for et in range(ET):
                    def mm(et=et, t_=t_, hp=hp):
                        nc.tensor.matmul(
                            t_, wk_sb[et][:, hp * 128:(hp + 1) * 128],
                            kin_sb[et][:, 512:1024],
                            start=(et == 0), stop=(et == ET - 1))
                    ops.append(mm)

                def evac(t_=t_, hp=hp):
                    nc.scalar.activation(kT[hp][1], t_, Act.Identity,
                                         bias=bk_sb[:, hp:hp + 1])
                ops.append(evac)
                return ops

            def v_sh1_ih1_burst(st):
                ops = []
                t_ = aux_tile()
                for et in range(ET):
                    def mm(et=et, t_=t_, st=st):
                        nc.tensor.matmul(
                            t_, vin_sb[et][:, 512 + st * 128:
                                           512 + (st + 1) * 128],
                            wv_sb[et][:, 512:1024],
                            start=(et == 0), stop=(et == ET - 1))
                    ops.append(mm)

                def evac(t_=t_, st=st):
                    nc.vector.tensor_tensor(
                        out=v65[4 + st][:, 8:16, 0:64],
                        in0=t_.rearrange("p (h d) -> p h d", h=8),
                        in1=bvb[:, 512:1024].rearrange("p (h d) -> p h d",
                                                       h=8),
                        op=Alu.add)
                ops.append(evac)
                return ops

            for op in k_sh1_burst(0):
                op()

            # ============ attention + fillers =============================
            # filler op queue: K-sh1 for hp 1..7 and V-sh1 ih=1, deadline
            # ordered; 2 ops drained per iteration.
            filler = []
            for item in ["K1", "K2", "V0", "K3", "V1", "K4", "V2", "K5",
                         "V3", "K6", "K7"]:
                if item.startswith("K"):
                    filler.extend(k_sh1_burst(int(item[1])))
                else:
                    filler.extend(v_sh1_ih1_burst(int(item[1])))
            # out-proj early accumulation for tt=0 (both ih) rides the
            # filler queue once the kv bursts drain (~gi 50); otn2[0..4]
            # are ready well before then.
            op_state = {}

            def oproj_mm1(tt, ih, et):
                if (tt, ih) not in op_state:
                    op_state[(tt, ih)] = psp.tile([128, 512], F32,
                                                  tag="aux", name="aux",
                                                  bufs=2)
                nc.tensor.matmul(
                    op_state[(tt, ih)],
                    otn2[et][:, tt * 128:(tt + 1) * 128],
                    wo_sb[et][:, ih * 512:(ih + 1) * 512],
                    start=(et == 0), stop=(et == ET - 1))

            for et in range(5):
                for ih in range(2):
                    filler.append(
                        (lambda et=et, ih=ih: oproj_mm1(0, ih, et)))
            filler.reverse()  # pop from end

            wo_sb = [None] * ET

            def dma_wo(et):
                w = wpool.tile([128, E], BF16, tag="w", name="w")
                nc.sync.dma_start(out=w, in_=wo_d[et * 128:(et + 1) * 128, :])
                wo_sb[et] = w

            bt_tiles = {}

            def dma_bias(gi):
                hp, j = gi // 8, gi % 8
                bt = btp.tile([128, 2, 512], BF16, tag="bt", name="bt")
                nc.sync.dma_start(
                    out=bt,
                    in_=bias_d[2 * hp:2 * hp + 2,
                               j * 128:(j + 1) * 128, :].transpose([1, 0, 2]))
                bt_tiles[gi] = bt

            for gi in range(3):
                dma_bias(gi)

            e_tiles = {}
            po_tiles = {}
            pending = []  # (hp, j) o_mms not yet emitted

            PE_ADD_J = ()  # identity-matmul path disabled (PE weight-buffer hazard)

            def emit_scores(gi):
                hp, j = gi // 8, gi % 8
                ps_j = pair_tile()
                sh, sl = j // 4, j % 4
                pe_add = j in PE_ADD_J
                for hh in range(2):
                    nc.tensor.matmul(
                        ps_j[:, hh, :],
                        kT[hp][sh][hh * 64:(hh + 1) * 64,
                                   sl * 128:(sl + 1) * 128],
                        qT[hp][hh * 64:(hh + 1) * 64, :],
                        start=True, stop=not pe_add,
                        tile_position=(hh * 64, 0))
                if pe_add:
                    bt = bt_tiles.pop(gi)
                    for hh in range(2):
                        nc.tensor.matmul(ps_j[:, hh, :], ident,
                                         bt[:, hh, :],
                                         start=False, stop=True)
                return ps_j

            def emit_add_exp(gi, ps_j):
                hp, j = gi // 8, gi % 8
                if j in PE_ADD_J:
                    src = ps_j
                else:
                    bt = bt_tiles.pop(gi)
                    ei = eip.tile([128, 2, 512], BF16, tag="ei", name="ei")
                    nc.vector.tensor_tensor(out=ei, in0=ps_j, in1=bt,
                                            op=Alu.add)
                    src = ei
                e_ = ep.tile([128, 2, 512], BF16, tag="e", name="e")
                nc.scalar.activation(e_, src, Act.Exp,
                                     bias=mask_sb[:, j:j + 1])
                e_tiles[gi] = e_

            def emit_o_mm(gi):
                hp, j = gi // 8, gi % 8
                if j == 0:
                    po_tiles[hp] = (
                        psp.tile([65, 512], F32, tag="ot0", name="ot0",
                                 bufs=1),
                        psp.tile([65, 512], F32, tag="ot1", name="ot1",
                                 bufs=1))
                e_ = e_tiles.pop(gi)
                for hh in range(2):
                    nc.tensor.matmul(po_tiles[hp][hh],
                                     v65[j][:, 2 * hp + hh, :],
                                     e_[:, hh, :],
                                     start=(j == 0), stop=(j == 7))

            def normalize(hp):
                # Evacuate both po banks to SBUF immediately (releases the
                # ot PSUM banks ~1.5us after the last o_mm instead of after
                # the whole DRAM-roundtrip chain), then normalize off-PSUM.
                po0, po1 = po_tiles.pop(hp)
                poS = small.tile([65, 2, 512], F32, tag="poS", name="poS",
                                 bufs=2)
                nc.scalar.copy(poS[:, 0, :], po0)
                nc.scalar.copy(poS[:, 1, :], po1)
                rcd = dramp.tile([1, 2, 512], F32, tag="rcd", name="rcd")
                nc.sync.dma_start(out=rcd, in_=poS[64:65, :, :])
                den2 = small.tile([2, 512], F32, tag="den2", name="den2",
                                  bufs=1)
                nc.sync.dma_start(out=den2, in_=rcd)
                rcpf = small.tile([2, 512], F32, tag="rcpf", name="rcpf",
                                  bufs=1)
                nc.vector.reciprocal_approx_fast(out=rcpf, in_=den2)
                rcd2 = dramp.tile([1, 2, 512], F32, tag="rcd2", name="rcd2")
                nc.sync.dma_start(out=rcd2, in_=rcpf)
                bcb = small.tile([64, 2, 512], F32, tag="bcb", name="bcb", bufs=1)
                nc.sync.dma_start(out=bcb[:, 0, :],
                                  in_=_pbcast(rcd2[:, 0, :], 64))
                nc.sync.dma_start(out=bcb[:, 1, :],
                                  in_=_pbcast(rcd2[:, 1, :], 64))
                nc.vector.tensor_tensor(out=otn2[hp][0:64, :],
                                        in0=poS[0:64, 0, :],
                                        in1=bcb[:, 0, :], op=Alu.mult)
                tmpn = small.tile([64, 512], BF16, tag="tmpn", name="tmpn", bufs=1)
                nc.vector.tensor_tensor(out=tmpn, in0=poS[0:64, 1, :],
                                        in1=bcb[:, 1, :], op=Alu.mult)
                nc.scalar.copy(otn2[hp][64:128, :], tmpn)

            for gi in range(64):
                if gi + 3 < 64:
                    dma_bias(gi + 3)
                if gi % 8 == 2 and gi // 8 < ET:
                    dma_wo(gi // 8)
                ps_j = emit_scores(gi)
                for _ in range(2):
                    if filler:
                        filler.pop()()
                emit_add_exp(gi, ps_j)
                pending.append(gi)
                # drain o_mms: keep lag >= 2; first two j of an hp wait
                # until gi reaches hp*8+4 so normalize(hp-1) can finish
                # reading the ot banks before they are reused.
                while pending:
                    og = pending[0]
                    if gi - og < 2:
                        break
                    if og % 8 < 2 and gi < (og // 8) * 8 + 5:
                        break
                    pending.pop(0)
                    emit_o_mm(og)
                    if og % 8 == 7:
                        normalize(og // 8)
            while filler:
                filler.pop()()

            def drain_upto(max_og):
                while pending and pending[0] <= max_og:
                    og = pending.pop(0)
                    emit_o_mm(og)
                    if og % 8 == 7:
                        normalize(og // 8)

            def oproj_evac(ps_o, tt, ih):
                o = osbp.tile([128, 512], F32, tag="osb", name="osb")
                nc.vector.tensor_tensor(
                    out=o, in0=ps_o,
                    in1=bob[:, ih * 512:(ih + 1) * 512], op=Alu.add)
                nc.sync.dma_start(
                    out=out_d[tt * 128:(tt + 1) * 128,
                              ih * 512:(ih + 1) * 512],
                    in_=o)

            # ===== out-proj tail: tt0 (aux) got ets 0..4 as fillers; run
            # its ets 5..6, drain the final o_mms + normalize(7), then put
            # tt1/tt2 et0..6 on the freed pair banks while the norm-7
            # chain completes; et7 groups and evacs last. ==============
            for ih in range(2):
                oproj_mm1(0, ih, 5)
                oproj_mm1(0, ih, 6)
            # tt1/tt2 et0..6 go on the pair banks BEFORE the final drain so
            # the PE has ~7us of queued work while the last adds/exps and
            # the normalize(7) chain complete on DVE/ACT.
            pt = {1: pair_tile(), 2: pair_tile()}
            for tt in (1, 2):
                for et in range(7):
                    for ih in range(2):
                        nc.tensor.matmul(
                            pt[tt][:, ih, :],
                            otn2[et][:, tt * 128:(tt + 1) * 128],
                            wo_sb[et][:, ih * 512:(ih + 1) * 512],
                            start=(et == 0), stop=False)
            drain_upto(63)               # o_mm(62,63) + normalize(7)
            for ih in range(2):
                oproj_mm1(0, ih, 7)
                oproj_evac(op_state.pop((0, ih)), 0, ih)
            for tt in (1, 2):
                for ih in range(2):
                    nc.tensor.matmul(
                        pt[tt][:, ih, :],
                        otn2[7][:, tt * 128:(tt + 1) * 128],
                        wo_sb[7][:, ih * 512:(ih + 1) * 512],
                        start=False, stop=True)
                for ih in range(2):
                    oproj_evac(pt[tt][:, ih, :], tt, ih)
            for ih in range(2):
                for et in range(ET):
                    oproj_mm1(3, ih, et)
                oproj_evac(op_state.pop((3, ih)), 3, ih)

    nc.compile()
    return nc


def _prepare_in_maps(query, key, value, key_padding_mask, attn_bias,
                     wq, bq, wk, bk, wv, bv, wo, bo):
    wqt = (np.ascontiguousarray(wq.T) * SCALING).astype(NPBF16)
    wkt = np.ascontiguousarray(wk.T).astype(NPBF16)
    wvt = np.ascontiguousarray(wv.T).astype(NPBF16)
    wot = np.ascontiguousarray(wo.T).astype(NPBF16)
    bqs = np.ascontiguousarray((bq * SCALING).reshape(8, 128).T)
    bks = np.ascontiguousarray(bk.astype(np.float32).reshape(8, 128).T)
    bvr = np.ascontiguousarray(bv).astype(NPBF16)[None, :]
    bor = np.ascontiguousarray(bo).astype(NPBF16)[None, :]

    ident = np.eye(128, dtype=NPBF16)
    kin_b = [np.ascontiguousarray(key[b_].T).astype(NPBF16) for b_ in range(B)]
    vin_b = [np.ascontiguousarray(value[b_].T).astype(NPBF16) for b_ in range(B)]
    maskneg_b = [
        np.ascontiguousarray(
            np.where(key_padding_mask[b_], MASK_NEG, 0.0)
            .astype(np.float32).reshape(8, 128).T)
        for b_ in range(B)
    ]

    in_maps = []
    for c in range(8):
        b_, th = c // 2, c % 2
        qin = np.ascontiguousarray(
            query[b_, th * TS:(th + 1) * TS, :].T).astype(NPBF16)
        biasT = np.ascontiguousarray(
            attn_bias[b_ * H:(b_ + 1) * H, th * TS:(th + 1) * TS, :]
            .transpose(0, 2, 1)).astype(NPBF16)
        in_maps.append({
            "qin": qin, "kin": kin_b[b_], "vin": vin_b[b_],
            "biasT": biasT, "maskneg": maskneg_b[b_],
            "wqt": wqt, "wkt": wkt, "wvt": wvt, "wot": wot,
            "bqs": bqs, "bks": bks, "bvr": bvr, "bor": bor, "ident": ident,
        })
    return in_maps


def kernel(query, key, value, key_padding_mask, attn_bias,
           wq, bq, wk, bk, wv, bv, wo, bo, _run_kwargs=None):
    query = np.asarray(query, dtype=np.float32)
    key = np.asarray(key, dtype=np.float32)
    value = np.asarray(value, dtype=np.float32)
    key_padding_mask = np.asarray(key_padding_mask)
    attn_bias = np.asarray(attn_bias, dtype=np.float32)
    wq, bq = np.asarray(wq, np.float32), np.asarray(bq, np.float32)
    wk, bk = np.asarray(wk, np.float32), np.asarray(bk, np.float32)
    wv, bv = np.asarray(wv, np.float32), np.asarray(bv, np.float32)
    wo, bo = np.asarray(wo, np.float32), np.asarray(bo, np.float32)

    if "nc" not in _CACHE:
        _CACHE["nc"] = build_nc()
    nc = _CACHE["nc"]

    in_maps = _prepare_in_maps(query, key, value, key_padding_mask, attn_bias,
                               wq, bq, wk, bk, wv, bv, wo, bo)
    res = run_bass_kernel_spmd(nc, in_maps, core_ids=list(range(8)),
                               **(_run_kwargs or {}))
    _CACHE["last_results"] = res

    out = np.empty((B, T, E), dtype=np.float32)
    for c in range(8):
        b_, th = c // 2, c % 2
        out[b_, th * TS:(th + 1) * TS, :] = res.results[c]["out"]
    return out
